# revision 27
# baseline (speedup 1.0000x reference)
"""Trainium2 Bass kernel for nn_DecoderCell (dense_transformer decoder cell).

Strategy
--------
Data parallel: batch B=256 sharded 32-per-core across 8 NeuronCores.

Algebraic reformulation (avoids materializing K1/V/K2, a ~20x FLOP cut):
  Q1[b]      = graph[b] @ Wq_fixed + step[b] @ Wq_step                  [128]
  T[b][:,h]  = (Wk1/sqrt(dh))[:, hs] @ Q1[b][hs]                        [128, 8]
  scores     = X[b] @ T[b]  (+ -60000 mask bias)                        [N, 8]
  E = exp(scores);  Z[h] = sum_n E;  s_un[h] = E[:,h]^T @ X[b]
  s = s_un / Z
  Q2 = sum_h s[h] @ Mc[h],   Mc[h] = Wv[:, hs] @ Wout[hs, :]
  r  = (Wk2/sqrt(D)) @ Q2
  out = 10*tanh(X[b] @ r)  masked to -inf

All big matmuls are bf16 with f32 PSUM accumulation. X is sent to HBM in
both natural [N,128] layout (n-contraction operand) and transposed
[128,N] layout (d-contraction operand); N padded 2000->2048.
"""
import sys

sys.path.insert(0, "/opt/trn_rl_repo")

import numpy as np
import ml_dtypes

B, N, D, H, DH = 256, 2000, 128, 8, 16
NPAD = 2048
NT = NPAD // 128  # 16
NCORES = 8
BL = B // NCORES  # 32
GROUP = 4         # batch elements per DMA batch / output batch
CLIP = 10.0
MASK_NEG = -60000.0
BF16 = ml_dtypes.bfloat16

_CACHE = {}


def _build_bass():
    import concourse.bass as bass
    import concourse.mybir as mybir
    from concourse import bacc
    from concourse.tile import TileContext

    f32 = mybir.dt.float32
    bf16 = mybir.dt.bfloat16
    Alu = mybir.AluOpType
    Act = mybir.ActivationFunctionType

    nc = bacc.Bacc()

    NG = BL // GROUP
    # batched layouts: GROUP batch elements per DMA, 16KB contiguous per
    # partition, to amortize per-descriptor overhead on the DMA queues
    xt = nc.declare_dram_parameter("xt", [NG, D, GROUP, NPAD], bf16,
                                   isOutput=False)
    xn = nc.declare_dram_parameter("xn", [NG, 128, GROUP, NT, D], bf16,
                                   isOutput=False)
    mb = nc.declare_dram_parameter("mb", [128, BL, NT], bf16, isOutput=False)
    mi = nc.declare_dram_parameter("mi", [128, BL, NT], f32, isOutput=False)
    gT = nc.declare_dram_parameter("gT", [D, BL], f32, isOutput=False)
    sTa = nc.declare_dram_parameter("sTa", [D, BL], f32, isOutput=False)
    sTb = nc.declare_dram_parameter("sTb", [2, BL], f32, isOutput=False)
    wqf = nc.declare_dram_parameter("wqf", [D, D], f32, isOutput=False)
    wqsa = nc.declare_dram_parameter("wqsa", [D, D], f32, isOutput=False)
    wqsb = nc.declare_dram_parameter("wqsb", [2, D], f32, isOutput=False)
    wk1tb = nc.declare_dram_parameter("wk1tb", [H, D, D], f32, isOutput=False)
    wvtb = nc.declare_dram_parameter("wvtb", [H, D, D], f32, isOutput=False)
    wout = nc.declare_dram_parameter("wout", [D, D], f32, isOutput=False)
    wk2t = nc.declare_dram_parameter("wk2t", [D, D], bf16, isOutput=False)
    out = nc.declare_dram_parameter("out", [NG, 128, GROUP, NT], f32,
                                    isOutput=True)

    with TileContext(nc) as tc:
        with tc.tile_pool(name="persist", bufs=1) as pers:
            # persistent SBUF: per-batch T (bf16), Mc blocks, Wk2T, ones
            t_sb = pers.tile([D, H, BL], bf16)
            mc_sb = pers.tile([D, H, D], bf16)
            wk2t_sb = pers.tile([D, D], bf16)
            ones_bf = pers.tile([128, 1], bf16)
            ones_f = pers.tile([1, 128], f32)
            mb_sb = pers.tile([128, BL, NT], bf16)
            mi_sb = pers.tile([128, BL, NT], f32)
            nc.vector.memset(ones_bf, 1.0)
            nc.vector.memset(ones_f, 1.0)
            nc.scalar.dma_start(out=wk2t_sb, in_=wk2t[:])
            nc.scalar.dma_start(out=mb_sb, in_=mb[:])
            nc.scalar.dma_start(out=mi_sb, in_=mi[:])

            # ---- prologue: Q1 -> T, Mc (once per core) ----
            with (
                tc.tile_pool(name="pro", bufs=1) as pro,
                tc.tile_pool(name="prop", bufs=1, space="PSUM") as prop,
            ):
                g_sb = pro.tile([D, BL], f32)
                sa_sb = pro.tile([D, BL], f32)
                sb_sb = pro.tile([2, BL], f32)
                wqf_sb = pro.tile([D, D], f32)
                wqsa_sb = pro.tile([D, D], f32)
                wqsb_sb = pro.tile([2, D], f32)
                wk1_sb = pro.tile([D, H, D], f32, padded_shape=None)
                wvt_sb = pro.tile([D, H, D], f32, padded_shape=None)
                wout_sb = pro.tile([D, D], f32)
                nc.scalar.dma_start(out=g_sb, in_=gT[:])
                nc.scalar.dma_start(out=sa_sb, in_=sTa[:])
                nc.scalar.dma_start(out=sb_sb, in_=sTb[:])
                nc.scalar.dma_start(out=wqf_sb, in_=wqf[:])
                nc.scalar.dma_start(out=wqsa_sb, in_=wqsa[:])
                nc.scalar.dma_start(out=wqsb_sb, in_=wqsb[:])
                nc.scalar.dma_start(out=wk1_sb,
                                  in_=wk1tb[:].rearrange("h j d -> j h d"))
                nc.scalar.dma_start(out=wvt_sb,
                                  in_=wvtb[:].rearrange("h j d -> j h d"))
                nc.scalar.dma_start(out=wout_sb, in_=wout[:])

                qp = prop.tile([D, BL], f32)
                nc.tensor.matmul(qp[:], lhsT=wqf_sb[:], rhs=g_sb[:],
                                 start=True, stop=False)
                nc.tensor.matmul(qp[:], lhsT=wqsa_sb[:], rhs=sa_sb[:],
                                 start=False, stop=False)
                nc.tensor.matmul(qp[:], lhsT=wqsb_sb[:], rhs=sb_sb[:],
                                 start=False, stop=True)
                q1_sb = pro.tile([D, BL], f32)
                nc.vector.tensor_copy(q1_sb[:], qp[:])

                tp = prop.tile([D, H, BL], f32)
                mcp = prop.tile([D, H, D], f32)
                for h in range(H):
                    nc.tensor.matmul(tp[:, h, :], lhsT=wk1_sb[:, h, :],
                                     rhs=q1_sb[:], start=True, stop=True)
                    nc.tensor.matmul(mcp[:, h, :], lhsT=wvt_sb[:, h, :],
                                     rhs=wout_sb[:], start=True, stop=True)
                nc.vector.tensor_copy(t_sb[:], tp[:])
                nc.vector.tensor_copy(mc_sb[:], mcp[:])

            # ---- main loop over batch elements ----
            with (
                tc.tile_pool(name="xtp", bufs=4) as xtp,
                tc.tile_pool(name="xnp", bufs=4) as xnp,
                tc.tile_pool(name="sm", bufs=4) as sm,
                tc.tile_pool(name="op", bufs=2) as op,
                tc.tile_pool(name="ps_sc", bufs=3, space="PSUM") as ps_sc,
                tc.tile_pool(name="ps_sz", bufs=3, space="PSUM") as ps_sz,
                tc.tile_pool(name="ps_lg", bufs=2, space="PSUM") as ps_lg,
            ):
                for g in range(NG):
                    og_sb = op.tile([128, GROUP, NT], f32, tag="og")
                    # batched loads: GROUP batch elements per dma_start
                    xt_t = xtp.tile([D, GROUP, NPAD], bf16, tag="xt")
                    nc.sync.dma_start(out=xt_t[:], in_=xt[g])
                    xn_t = xnp.tile([128, GROUP, NT, D], bf16, tag="xn")
                    nc.gpsimd.dma_start(out=xn_t[:], in_=xn[g])
                    for j in range(GROUP):
                        b = g * GROUP + j

                        # scores.T tiles: [128n, NT, H]
                        scp = ps_sc.tile([128, NT, H], f32, tag="sc")
                        for t in range(NT):
                            nc.tensor.matmul(
                                scp[:, t, :],
                                lhsT=xt_t[:, j, t * 128:(t + 1) * 128],
                                rhs=t_sb[:, :, b],
                                start=True, stop=True)
                        # mask-bias add (broadcast over h)
                        mb_b = mb_sb[:, b, :]
                        mb_bc = bass.AP(
                            tensor=mb_b.tensor, offset=mb_b.offset,
                            ap=[list(mb_b.ap[0]), list(mb_b.ap[1]), [0, H]])
                        msc = sm.tile([128, NT, H], f32, tag="msc")
                        nc.vector.tensor_tensor(
                            out=msc[:], in0=scp[:], in1=mb_bc, op=Alu.add)
                        e_sb = sm.tile([128, NT, H], bf16, tag="e")
                        nc.scalar.activation(e_sb[:], msc[:], Act.Exp)

                        # s_un + Z + tail, all in one PSUM bank:
                        # [0:H]=s_un  [H:H+128]=Z partials  [H+128:H+136]=1/Z rep
                        # [H+136]=Q2  [H+137]=r
                        sz = ps_sz.tile([128, H + 138], f32, tag="sz")
                        # Z first so the DVE reduce/recip chain overlaps s-mms
                        e_flat = e_sb.rearrange("p t h -> p (t h)")
                        nc.tensor.matmul(sz[0:1, H:H + 128], lhsT=ones_bf[:],
                                         rhs=e_flat, start=True, stop=True)
                        for t in range(NT):
                            nc.tensor.matmul(
                                sz[:, 0:H],
                                lhsT=xn_t[:, j, t, :],
                                rhs=e_sb[:, t, :],
                                start=(t == 0), stop=(t == NT - 1))
                        zin = sz[0:1, H:H + 128].rearrange(
                            "p (t h) -> p h t", h=H)
                        zred = sm.tile([1, H], f32, tag="zr")
                        nc.vector.reduce_sum(out=zred[:], in_=zin,
                                             axis=mybir.AxisListType.X)
                        rz = sm.tile([1, H], f32, tag="rz")
                        nc.vector.reciprocal(rz[:], zred[:])
                        # replicate 1/Z across partitions via PE, then s = s_un/Z
                        nc.tensor.matmul(sz[:, H + 128:H + 136], lhsT=ones_f[:],
                                         rhs=rz[:], start=True, stop=True)
                        rep_sb = sm.tile([128, H], f32, tag="rep")
                        nc.vector.tensor_copy(rep_sb[:], sz[:, H + 128:H + 136])
                        s_sb = sm.tile([128, H], bf16, tag="s")
                        nc.vector.tensor_mul(s_sb[:], sz[:, 0:H], rep_sb[:])

                        # Q2 = sum_h s[h] @ Mc[h]  (lhsT=Mc[h], rhs=s[:,h])
                        for h in range(H):
                            nc.tensor.matmul(
                                sz[:, H + 136:H + 137], lhsT=mc_sb[:, h, :],
                                rhs=s_sb[:, h:h + 1],
                                start=(h == 0), stop=(h == H - 1))
                        q2_sb = sm.tile([128, 1], bf16, tag="q2")
                        nc.vector.tensor_copy(q2_sb[:], sz[:, H + 136:H + 137])
                        nc.tensor.matmul(sz[:, H + 137:H + 138], lhsT=wk2t_sb[:],
                                         rhs=q2_sb[:], start=True, stop=True)
                        r_sb = sm.tile([128, 1], bf16, tag="r")
                        nc.vector.tensor_copy(r_sb[:], sz[:, H + 137:H + 138])

                        # logits pass
                        lgp = ps_lg.tile([128, NT], f32, tag="lg")
                        for t in range(NT):
                            nc.tensor.matmul(
                                lgp[:, t:t + 1],
                                lhsT=xt_t[:, j, t * 128:(t + 1) * 128],
                                rhs=r_sb[:], start=True, stop=True)
                        th_sb = sm.tile([128, NT], f32, tag="th")
                        nc.scalar.activation(th_sb[:], lgp[:], Act.Tanh)
                        nc.vector.scalar_tensor_tensor(
                            out=og_sb[:, j, :], in0=th_sb[:], scalar=CLIP,
                            in1=mi_sb[:, b, :], op0=Alu.mult, op1=Alu.add)
                    nc.scalar.dma_start(out=out[g], in_=og_sb[:])
    nc.compile()
    return nc


def _get_nc():
    if "nc" not in _CACHE:
        _CACHE["nc"] = _build_bass()
    return _CACHE["nc"]


def _prep_host(inputs):
    X = np.asarray(inputs["node_embeddings"], np.float32)
    g = np.asarray(inputs["graph_embedding"], np.float32)
    sc = np.asarray(inputs["step_context"], np.float32)[:, 0, :]
    mask = np.asarray(inputs["mask"])[:, :, 0].astype(bool)
    Wk1 = np.asarray(inputs["Wk1"], np.float32)
    Wv = np.asarray(inputs["Wv"], np.float32)
    Wk2 = np.asarray(inputs["Wk2"], np.float32)
    Wqf = np.asarray(inputs["Wq_fixed"], np.float32)
    Wout = np.asarray(inputs["Wout"], np.float32)
    Wqs = np.asarray(inputs["Wq_step"], np.float32)

    Xp = np.zeros((B, NPAD, D), np.float32)
    Xp[:, :N] = X
    NGF = B // GROUP
    # natural X: [g][p][j][t][d] — per partition, GROUP*NT*D contiguous
    xn_h = np.ascontiguousarray(
        Xp.reshape(NGF, GROUP, NT, 128, D).transpose(0, 3, 1, 2, 4)
    ).astype(BF16)
    # transposed X: [g][d][j][n]
    xt_h = np.ascontiguousarray(
        Xp.reshape(NGF, GROUP, NPAD, D).transpose(0, 3, 1, 2)).astype(BF16)
    maskp = np.ones((B, NPAD), bool)
    maskp[:, :N] = mask
    mtiles = maskp.reshape(B, NT, 128).transpose(2, 0, 1)  # [128, B, NT]
    mb_h = np.where(mtiles, np.float32(MASK_NEG),
                    np.float32(0)).astype(BF16)
    mi_h = np.where(mtiles, np.float32(-np.inf),
                    np.float32(0)).astype(np.float32)

    # per-head blocks as full-K [j, d] transposed weights with rows outside
    # the head's 16-slice zeroed (avoids partition-offset matmuls on device)
    wk1s = (Wk1 / np.sqrt(DH)).astype(np.float32)
    wk1tb_h = np.zeros((H, D, D), np.float32)
    wvtb_h = np.zeros((H, D, D), np.float32)
    for h in range(H):
        hs = slice(h * DH, (h + 1) * DH)
        wk1tb_h[h, hs, :] = wk1s[:, hs].T
        wvtb_h[h, hs, :] = Wv[:, hs].T
    wk2t_h = np.ascontiguousarray(
        (Wk2 / np.sqrt(D)).T).astype(BF16)

    scT = np.ascontiguousarray(sc.T).astype(np.float32)  # [130, B]
    gT_full = np.ascontiguousarray(g.T).astype(np.float32)  # [128, B]

    NGC = BL // GROUP  # per-core group count
    in_maps = []
    for c in range(NCORES):
        bs = slice(c * BL, (c + 1) * BL)
        gsl = slice(c * NGC, (c + 1) * NGC)
        in_maps.append({
            "xt": np.ascontiguousarray(xt_h[gsl]),
            "xn": np.ascontiguousarray(xn_h[gsl]),
            "mb": np.ascontiguousarray(mb_h[:, bs]),
            "mi": np.ascontiguousarray(mi_h[:, bs]),
            "gT": np.ascontiguousarray(gT_full[:, bs]),
            "sTa": np.ascontiguousarray(scT[:D, bs]),
            "sTb": np.ascontiguousarray(scT[D:, bs]),
            "wqf": Wqf,
            "wqsa": np.ascontiguousarray(Wqs[:D]),
            "wqsb": np.ascontiguousarray(Wqs[D:]),
            "wk1tb": wk1tb_h,
            "wvtb": wvtb_h,
            "wout": Wout,
            "wk2t": wk2t_h,
        })
    return in_maps


def kernel(**inputs):
    from concourse.bass_utils import run_bass_kernel_spmd

    nc = _get_nc()
    in_maps = _prep_host(inputs)
    res = run_bass_kernel_spmd(nc, in_maps, core_ids=list(range(NCORES)))
    kernel._last = res
    outs = []
    for c in range(NCORES):
        o = np.asarray(res.results[c]["out"])  # [NG, 128, GROUP, NT]
        o = o.transpose(0, 2, 3, 1).reshape(BL, NPAD)[:, :N]
        outs.append(o)
    return np.concatenate(outs, 0).astype(np.float32)


kernel._last = None


# revision 31
# speedup vs baseline: 1.0964x; 1.0964x over previous
"""Trainium2 Bass kernel for nn_DecoderCell (dense_transformer decoder cell).

Strategy
--------
Data parallel: batch B=256 sharded 32-per-core across 8 NeuronCores.

Algebraic reformulation (avoids materializing K1/V/K2, a ~20x FLOP cut):
  Q1[b]      = graph[b] @ Wq_fixed + step[b] @ Wq_step                  [128]
  T[b][:,h]  = (Wk1/sqrt(dh))[:, hs] @ Q1[b][hs]                        [128, 8]
  scores     = X[b] @ T[b]  (+ -60000 mask bias)                        [N, 8]
  E = exp(scores);  Z[h] = sum_n E;  s_un[h] = E[:,h]^T @ X[b]
  s = s_un / Z
  Q2 = sum_h s[h] @ Mc[h],   Mc[h] = Wv[:, hs] @ Wout[hs, :]
  r  = (Wk2/sqrt(D)) @ Q2
  out = 10*tanh(X[b] @ r)  masked to -inf

All big matmuls are bf16 with f32 PSUM accumulation. X is sent to HBM in
both natural [N,128] layout (n-contraction operand) and transposed
[128,N] layout (d-contraction operand); N padded 2000->2048.
"""
import sys

sys.path.insert(0, "/opt/trn_rl_repo")

import numpy as np
import ml_dtypes

B, N, D, H, DH = 256, 2000, 128, 8, 16
NPAD = 2048
NT = NPAD // 128  # 16
NCORES = 8
BL = B // NCORES  # 32
GROUP = 4         # batch elements per DMA batch / output batch
CLIP = 10.0
MASK_NEG = -60000.0
BF16 = ml_dtypes.bfloat16

_CACHE = {}


def _build_bass():
    import concourse.bass as bass
    import concourse.mybir as mybir
    from concourse import bacc
    from concourse.tile import TileContext

    f32 = mybir.dt.float32
    bf16 = mybir.dt.bfloat16
    Alu = mybir.AluOpType
    Act = mybir.ActivationFunctionType

    nc = bacc.Bacc()

    NG = BL // GROUP
    # batched layouts: GROUP batch elements per DMA, 16KB contiguous per
    # partition, to amortize per-descriptor overhead on the DMA queues
    xt = nc.declare_dram_parameter("xt", [NG, D, GROUP, NPAD], bf16,
                                   isOutput=False)
    xn = nc.declare_dram_parameter("xn", [NG, 128, GROUP, NT, D], bf16,
                                   isOutput=False)
    mb = nc.declare_dram_parameter("mb", [128, BL, NT], bf16, isOutput=False)
    mi = nc.declare_dram_parameter("mi", [128, BL, NT], f32, isOutput=False)
    gT = nc.declare_dram_parameter("gT", [D, BL], f32, isOutput=False)
    sTa = nc.declare_dram_parameter("sTa", [D, BL], f32, isOutput=False)
    sTb = nc.declare_dram_parameter("sTb", [2, BL], f32, isOutput=False)
    wqf = nc.declare_dram_parameter("wqf", [D, D], f32, isOutput=False)
    wqsa = nc.declare_dram_parameter("wqsa", [D, D], f32, isOutput=False)
    wqsb = nc.declare_dram_parameter("wqsb", [2, D], f32, isOutput=False)
    wk1tb = nc.declare_dram_parameter("wk1tb", [H, D, D], f32, isOutput=False)
    wvtb = nc.declare_dram_parameter("wvtb", [H, D, D], f32, isOutput=False)
    wout = nc.declare_dram_parameter("wout", [D, D], f32, isOutput=False)
    wk2t = nc.declare_dram_parameter("wk2t", [D, D], bf16, isOutput=False)
    out = nc.declare_dram_parameter("out", [NG, 128, GROUP, NT], f32,
                                    isOutput=True)

    with TileContext(nc) as tc:
        with tc.tile_pool(name="persist", bufs=1) as pers:
            # persistent SBUF: per-batch T (bf16), Mc blocks, Wk2T, ones
            t_sb = pers.tile([D, H, BL], bf16)
            mc_sb = pers.tile([D, H, D], bf16)
            wk2t_sb = pers.tile([D, D], bf16)
            ones_bf = pers.tile([128, 128], bf16)
            mb_sb = pers.tile([128, BL, NT], bf16)
            mi_sb = pers.tile([128, BL, NT], f32)
            nc.vector.memset(ones_bf, 1.0)
            nc.scalar.dma_start(out=wk2t_sb, in_=wk2t[:])
            nc.scalar.dma_start(out=mb_sb, in_=mb[:])
            nc.scalar.dma_start(out=mi_sb, in_=mi[:])

            # ---- prologue: Q1 -> T, Mc (once per core) ----
            with (
                tc.tile_pool(name="pro", bufs=1) as pro,
                tc.tile_pool(name="prop", bufs=1, space="PSUM") as prop,
            ):
                g_sb = pro.tile([D, BL], f32)
                sa_sb = pro.tile([D, BL], f32)
                sb_sb = pro.tile([2, BL], f32)
                wqf_sb = pro.tile([D, D], f32)
                wqsa_sb = pro.tile([D, D], f32)
                wqsb_sb = pro.tile([2, D], f32)
                wk1_sb = pro.tile([D, H, D], f32, padded_shape=None)
                wvt_sb = pro.tile([D, H, D], f32, padded_shape=None)
                wout_sb = pro.tile([D, D], f32)
                nc.scalar.dma_start(out=g_sb, in_=gT[:])
                nc.scalar.dma_start(out=sa_sb, in_=sTa[:])
                nc.scalar.dma_start(out=sb_sb, in_=sTb[:])
                nc.scalar.dma_start(out=wqf_sb, in_=wqf[:])
                nc.scalar.dma_start(out=wqsa_sb, in_=wqsa[:])
                nc.scalar.dma_start(out=wqsb_sb, in_=wqsb[:])
                nc.scalar.dma_start(out=wk1_sb,
                                  in_=wk1tb[:].rearrange("h j d -> j h d"))
                nc.scalar.dma_start(out=wvt_sb,
                                  in_=wvtb[:].rearrange("h j d -> j h d"))
                nc.scalar.dma_start(out=wout_sb, in_=wout[:])

                qp = prop.tile([D, BL], f32)
                nc.tensor.matmul(qp[:], lhsT=wqf_sb[:], rhs=g_sb[:],
                                 start=True, stop=False)
                nc.tensor.matmul(qp[:], lhsT=wqsa_sb[:], rhs=sa_sb[:],
                                 start=False, stop=False)
                nc.tensor.matmul(qp[:], lhsT=wqsb_sb[:], rhs=sb_sb[:],
                                 start=False, stop=True)
                q1_sb = pro.tile([D, BL], f32)
                nc.vector.tensor_copy(q1_sb[:], qp[:])

                tp = prop.tile([D, H, BL], f32)
                mcp = prop.tile([D, H, D], f32)
                for h in range(H):
                    nc.tensor.matmul(tp[:, h, :], lhsT=wk1_sb[:, h, :],
                                     rhs=q1_sb[:], start=True, stop=True)
                    nc.tensor.matmul(mcp[:, h, :], lhsT=wvt_sb[:, h, :],
                                     rhs=wout_sb[:], start=True, stop=True)
                nc.vector.tensor_copy(t_sb[:], tp[:])
                nc.vector.tensor_copy(mc_sb[:], mcp[:])

            # ensure prologue (small weight DMAs + T/Mc) completes before the
            # bulk xt/xn prefetch storm monopolizes the DMA rings
            tc.strict_bb_all_engine_barrier()

            # ---- main loop over batch elements ----
            with (
                tc.tile_pool(name="xtp", bufs=4) as xtp,
                tc.tile_pool(name="xnp", bufs=4) as xnp,
                tc.tile_pool(name="sm", bufs=4) as sm,
                tc.tile_pool(name="op", bufs=2) as op,
                tc.tile_pool(name="ps_sc", bufs=3, space="PSUM") as ps_sc,
                tc.tile_pool(name="ps_sz", bufs=3, space="PSUM") as ps_sz,
                tc.tile_pool(name="ps_lg", bufs=2, space="PSUM") as ps_lg,
            ):
                for g in range(NG):
                    og_sb = op.tile([128, GROUP, NT], f32, tag="og")
                    # batched loads: GROUP batch elements per dma_start
                    xt_t = xtp.tile([D, GROUP, NPAD], bf16, tag="xt")
                    nc.sync.dma_start(out=xt_t[:], in_=xt[g])
                    xn_t = xnp.tile([128, GROUP, NT, D], bf16, tag="xn")
                    nc.gpsimd.dma_start(out=xn_t[:], in_=xn[g])
                    for j in range(GROUP):
                        b = g * GROUP + j

                        # scores.T tiles: [128n, NT, H]
                        scp = ps_sc.tile([128, NT, H], f32, tag="sc")
                        for t in range(NT):
                            nc.tensor.matmul(
                                scp[:, t, :],
                                lhsT=xt_t[:, j, t * 128:(t + 1) * 128],
                                rhs=t_sb[:, :, b],
                                start=True, stop=True)
                        # mask-bias add (broadcast over h)
                        mb_b = mb_sb[:, b, :]
                        mb_bc = bass.AP(
                            tensor=mb_b.tensor, offset=mb_b.offset,
                            ap=[list(mb_b.ap[0]), list(mb_b.ap[1]), [0, H]])
                        msc = sm.tile([128, NT, H], f32, tag="msc")
                        nc.vector.tensor_tensor(
                            out=msc[:], in0=scp[:], in1=mb_bc, op=Alu.add)
                        e_sb = sm.tile([128, NT, H], bf16, tag="e")
                        nc.scalar.activation(e_sb[:], msc[:], Act.Exp)

                        # s_un + Z + tail, all in one PSUM bank:
                        # [0:H]=s_un  [H:H+128]=Z partials (replicated on all
                        # partitions)  [H+128]=Q2  [H+129]=r
                        sz = ps_sz.tile([128, H + 130], f32, tag="sz")
                        # Z first so the DVE reduce/recip chain overlaps s-mms
                        e_flat = e_sb.rearrange("p t h -> p (t h)")
                        nc.tensor.matmul(sz[:, H:H + 128], lhsT=ones_bf[:],
                                         rhs=e_flat, start=True, stop=True,
                                         skip_group_check=True)
                        for t in range(NT):
                            nc.tensor.matmul(
                                sz[:, 0:H],
                                lhsT=xn_t[:, j, t, :],
                                rhs=e_sb[:, t, :],
                                start=(t == 0), stop=(t == NT - 1),
                                skip_group_check=True)
                        zin = sz[:, H:H + 128].rearrange(
                            "p (t h) -> p h t", h=H)
                        zred = sm.tile([128, H], f32, tag="zr")
                        nc.vector.reduce_sum(out=zred[:], in_=zin,
                                             axis=mybir.AxisListType.X)
                        rz = sm.tile([128, H], f32, tag="rz")
                        nc.vector.reciprocal(rz[:], zred[:])
                        s_sb = sm.tile([128, H], bf16, tag="s")
                        nc.vector.tensor_mul(s_sb[:], sz[:, 0:H], rz[:])

                        # Q2 = sum_h s[h] @ Mc[h]  (lhsT=Mc[h], rhs=s[:,h])
                        for h in range(H):
                            nc.tensor.matmul(
                                sz[:, H + 128:H + 129], lhsT=mc_sb[:, h, :],
                                rhs=s_sb[:, h:h + 1],
                                start=(h == 0), stop=(h == H - 1),
                                skip_group_check=True)
                        q2_sb = sm.tile([128, 1], bf16, tag="q2")
                        nc.vector.tensor_copy(q2_sb[:], sz[:, H + 128:H + 129])
                        nc.tensor.matmul(sz[:, H + 129:H + 130], lhsT=wk2t_sb[:],
                                         rhs=q2_sb[:], start=True, stop=True,
                                         skip_group_check=True)
                        r_sb = sm.tile([128, 1], bf16, tag="r")
                        nc.vector.tensor_copy(r_sb[:], sz[:, H + 129:H + 130])

                        # logits pass
                        lgp = ps_lg.tile([128, NT], f32, tag="lg")
                        for t in range(NT):
                            nc.tensor.matmul(
                                lgp[:, t:t + 1],
                                lhsT=xt_t[:, j, t * 128:(t + 1) * 128],
                                rhs=r_sb[:], start=True, stop=True)
                        th_sb = sm.tile([128, NT], f32, tag="th")
                        nc.scalar.activation(th_sb[:], lgp[:], Act.Tanh)
                        nc.vector.scalar_tensor_tensor(
                            out=og_sb[:, j, :], in0=th_sb[:], scalar=CLIP,
                            in1=mi_sb[:, b, :], op0=Alu.mult, op1=Alu.add)
                    nc.scalar.dma_start(out=out[g], in_=og_sb[:])
    nc.compile()
    return nc


def _get_nc():
    if "nc" not in _CACHE:
        _CACHE["nc"] = _build_bass()
    return _CACHE["nc"]


def _prep_host(inputs):
    X = np.asarray(inputs["node_embeddings"], np.float32)
    g = np.asarray(inputs["graph_embedding"], np.float32)
    sc = np.asarray(inputs["step_context"], np.float32)[:, 0, :]
    mask = np.asarray(inputs["mask"])[:, :, 0].astype(bool)
    Wk1 = np.asarray(inputs["Wk1"], np.float32)
    Wv = np.asarray(inputs["Wv"], np.float32)
    Wk2 = np.asarray(inputs["Wk2"], np.float32)
    Wqf = np.asarray(inputs["Wq_fixed"], np.float32)
    Wout = np.asarray(inputs["Wout"], np.float32)
    Wqs = np.asarray(inputs["Wq_step"], np.float32)

    Xp = np.zeros((B, NPAD, D), np.float32)
    Xp[:, :N] = X
    NGF = B // GROUP
    # natural X: [g][p][j][t][d] — per partition, GROUP*NT*D contiguous
    xn_h = np.ascontiguousarray(
        Xp.reshape(NGF, GROUP, NT, 128, D).transpose(0, 3, 1, 2, 4)
    ).astype(BF16)
    # transposed X: [g][d][j][n]
    xt_h = np.ascontiguousarray(
        Xp.reshape(NGF, GROUP, NPAD, D).transpose(0, 3, 1, 2)).astype(BF16)
    maskp = np.ones((B, NPAD), bool)
    maskp[:, :N] = mask
    mtiles = maskp.reshape(B, NT, 128).transpose(2, 0, 1)  # [128, B, NT]
    mb_h = np.where(mtiles, np.float32(MASK_NEG),
                    np.float32(0)).astype(BF16)
    mi_h = np.where(mtiles, np.float32(-np.inf),
                    np.float32(0)).astype(np.float32)

    # per-head blocks as full-K [j, d] transposed weights with rows outside
    # the head's 16-slice zeroed (avoids partition-offset matmuls on device)
    wk1s = (Wk1 / np.sqrt(DH)).astype(np.float32)
    wk1tb_h = np.zeros((H, D, D), np.float32)
    wvtb_h = np.zeros((H, D, D), np.float32)
    for h in range(H):
        hs = slice(h * DH, (h + 1) * DH)
        wk1tb_h[h, hs, :] = wk1s[:, hs].T
        wvtb_h[h, hs, :] = Wv[:, hs].T
    wk2t_h = np.ascontiguousarray(
        (Wk2 / np.sqrt(D)).T).astype(BF16)

    scT = np.ascontiguousarray(sc.T).astype(np.float32)  # [130, B]
    gT_full = np.ascontiguousarray(g.T).astype(np.float32)  # [128, B]

    NGC = BL // GROUP  # per-core group count
    in_maps = []
    for c in range(NCORES):
        bs = slice(c * BL, (c + 1) * BL)
        gsl = slice(c * NGC, (c + 1) * NGC)
        in_maps.append({
            "xt": np.ascontiguousarray(xt_h[gsl]),
            "xn": np.ascontiguousarray(xn_h[gsl]),
            "mb": np.ascontiguousarray(mb_h[:, bs]),
            "mi": np.ascontiguousarray(mi_h[:, bs]),
            "gT": np.ascontiguousarray(gT_full[:, bs]),
            "sTa": np.ascontiguousarray(scT[:D, bs]),
            "sTb": np.ascontiguousarray(scT[D:, bs]),
            "wqf": Wqf,
            "wqsa": np.ascontiguousarray(Wqs[:D]),
            "wqsb": np.ascontiguousarray(Wqs[D:]),
            "wk1tb": wk1tb_h,
            "wvtb": wvtb_h,
            "wout": Wout,
            "wk2t": wk2t_h,
        })
    return in_maps


def kernel(**inputs):
    from concourse.bass_utils import run_bass_kernel_spmd

    nc = _get_nc()
    in_maps = _prep_host(inputs)
    res = run_bass_kernel_spmd(nc, in_maps, core_ids=list(range(NCORES)))
    kernel._last = res
    outs = []
    for c in range(NCORES):
        o = np.asarray(res.results[c]["out"])  # [NG, 128, GROUP, NT]
        o = o.transpose(0, 2, 3, 1).reshape(BL, NPAD)[:, :N]
        outs.append(o)
    return np.concatenate(outs, 0).astype(np.float32)


kernel._last = None
